# revision 3
# baseline (speedup 1.0000x reference)
"""Trainium2 Bass kernel for nn_Attention_37641093382387.

Dense transformer attention block:
  qkv = x @ Wqkv; q,k + RoPE; causal softmax attention; out @ Wproj + bproj

Sharding: 8 cores = 2 batches x 4 head-groups (4 heads each).  Each core
computes its batch's partial output for its head group; host sums the 4
group partials per batch and adds the bias.

Per-core device pipeline (all matmuls bf16 -> f32 PSUM):
  - host passes x^T (pre-transposed, bf16) so no on-chip transposes needed
  - qT,kT computed in [hd, T] layout (lhsT=W block, rhs=xT);
    v in [T, hd] layout (lhsT=xT block, rhs=Wv)
  - RoPE rotate-half done via a +-1 permutation matmul on PE (DVE has no
    cross-partition path), then 3 DVE ops
  - attention computed transposed: ST[s,t] = kT_tile^T @ qT -> exp on ACT
    (scale folded into exp) -> PT bf16; row sums via all-ones matmul
    (replicated across partitions); OT accum = v_tile^T @ PT;
    normalization via DVE reciprocal+mul; causal handled by narrowing
    matmuls to the valid t-range + one 128x128 triangle mask multiply
  - proj: Y = OT^T blocks @ Wproj, f32 out
"""

import os
import sys

import numpy as np

for _p in ("/opt/trn_rl_repo",):
    if _p not in sys.path and os.path.isdir(_p):
        sys.path.insert(0, _p)

import ml_dtypes

bf16 = ml_dtypes.bfloat16

P = 128
T = 2048
D = 2048
HD = 128
NG = 4      # head groups
HPG = 4     # heads per group
B = 2
BK = 512    # t block
NB = T // BK          # 4 t-blocks
NKT = D // P          # 16 contraction chunks
NTT = T // P          # 16 t-tiles
SCALE = float(HD) ** -0.5

_NC_CACHE = {}


def _build_nc():
    import concourse.mybir as mybir
    from concourse import bacc
    from concourse.tile import TileContext

    fp32 = mybir.dt.float32
    bf = mybir.dt.bfloat16
    Exp = mybir.ActivationFunctionType.Exp

    nc = bacc.Bacc("TRN2", target_bir_lowering=False, debug=False,
                   num_devices=B * NG)

    xt_d = nc.declare_dram_parameter("xt", [D, T], bf, isOutput=False)
    wqk_d = nc.declare_dram_parameter("wqk", [D, 2 * HPG * HD], bf, isOutput=False)
    wv_d = nc.declare_dram_parameter("wv", [D, HPG * HD], bf, isOutput=False)
    wp_d = nc.declare_dram_parameter("wp", [HPG * HD, D], bf, isOutput=False)
    cos_d = nc.declare_dram_parameter("cos", [HD, T], bf, isOutput=False)
    sin_d = nc.declare_dram_parameter("sin", [HD, T], bf, isOutput=False)
    rot_d = nc.declare_dram_parameter("rot", [HD, HD], bf, isOutput=False)
    tri_d = nc.declare_dram_parameter("tri", [P, P], bf, isOutput=False)
    ones_d = nc.declare_dram_parameter("ones", [P, P], bf, isOutput=False)
    out_d = nc.declare_dram_parameter("out", [T, D], fp32, isOutput=True)

    xt_r = xt_d[:].rearrange("(kt p) t -> p kt t", p=P)

    with TileContext(nc) as tc, \
         tc.tile_pool(name="const", bufs=1) as constp, \
         tc.tile_pool(name="persist", bufs=1) as persistp, \
         tc.tile_pool(name="xt", bufs=2) as xtp, \
         tc.tile_pool(name="qblk", bufs=2) as qp, \
         tc.tile_pool(name="otblk", bufs=2) as otp, \
         tc.tile_pool(name="work", bufs=3) as workp, \
         tc.tile_pool(name="pt", bufs=4) as ptp, \
         tc.tile_pool(name="psmm", bufs=2, space="PSUM") as psmm, \
         tc.tile_pool(name="psrot", bufs=2, space="PSUM") as psrot, \
         tc.tile_pool(name="pssum", bufs=2, space="PSUM") as pssum, \
         tc.tile_pool(name="pso", bufs=2, space="PSUM") as psop:

        # ---- constants ----
        wqk_sb = constp.tile([P, NKT, 2 * HPG * HD], bf)
        nc.sync.dma_start(wqk_sb[:], wqk_d[:].rearrange("(kt p) e -> p kt e", p=P))
        wv_sb = constp.tile([P, NKT, HPG * HD], bf)
        nc.sync.dma_start(wv_sb[:], wv_d[:].rearrange("(kt p) e -> p kt e", p=P))
        wp_sb = constp.tile([P, HPG, D], bf)
        nc.sync.dma_start(wp_sb[:], wp_d[:].rearrange("(ct p) f -> p ct f", p=P))
        cos_sb = constp.tile([HD, T], bf)
        nc.sync.dma_start(cos_sb[:], cos_d[:])
        sin_sb = constp.tile([HD, T], bf)
        nc.sync.dma_start(sin_sb[:], sin_d[:])
        rot_sb = constp.tile([HD, HD], bf)
        nc.sync.dma_start(rot_sb[:], rot_d[:])
        tri_sb = constp.tile([P, P], bf)
        nc.sync.dma_start(tri_sb[:], tri_d[:])
        ones_sb = constp.tile([P, P], bf)
        nc.sync.dma_start(ones_sb[:], ones_d[:])

        # ---- persistent tensors ----
        k_sb = persistp.tile([HD, HPG, T], bf)        # kT per head
        v_sb = persistp.tile([P, NTT, HPG * HD], bf)  # v  per t-tile

        for j in range(NB):
            tsl = slice(j * BK, (j + 1) * BK)

            # ================= Phase Q(j): qkT + RoPE, v =================
            xt_sb = xtp.tile([P, NKT, BK], bf, tag="xt")
            nc.sync.dma_start(xt_sb[:], xt_r[:, :, tsl])

            q_sb = qp.tile([HD, HPG, BK], bf, tag="qblk")

            for e in range(2 * HPG):
                ps = psmm.tile([P, BK], fp32, tag="mm")
                for kt in range(NKT):
                    nc.tensor.matmul(
                        ps[:],
                        wqk_sb[:, kt, e * HD:(e + 1) * HD],
                        xt_sb[:, kt, :],
                        start=(kt == 0), stop=(kt == NKT - 1),
                    )
                # rotate-half via permutation matmul (needs bf16 SBUF copy)
                raw = workp.tile([P, BK], bf, tag="raw")
                nc.vector.tensor_copy(raw[:], ps[:])
                psr = psrot.tile([P, BK], fp32, tag="rot")
                nc.tensor.matmul(psr[:], rot_sb[:], raw[:], start=True, stop=True)
                t1 = workp.tile([P, BK], fp32, tag="t1")
                nc.vector.tensor_mul(t1[:], ps[:], cos_sb[:, tsl])
                t2 = workp.tile([P, BK], fp32, tag="t2")
                nc.vector.tensor_mul(t2[:], psr[:], sin_sb[:, tsl])
                if e < HPG:
                    dst = q_sb[:, e, :]
                else:
                    dst = k_sb[:, e - HPG, tsl]
                nc.vector.tensor_add(dst, t1[:], t2[:])

            for tt in range(BK // P):
                ps = psmm.tile([P, BK], fp32, tag="mm")
                for kt in range(NKT):
                    nc.tensor.matmul(
                        ps[:],
                        xt_sb[:, kt, tt * P:(tt + 1) * P],
                        wv_sb[:, kt, :],
                        start=(kt == 0), stop=(kt == NKT - 1),
                    )
                nc.vector.tensor_copy(v_sb[:, 4 * j + tt, :], ps[:])

            # ================= Phase A(j): attention =================
            ot_sb = otp.tile([HD, HPG, BK], bf, tag="otblk")
            ni = 4 * j + 4
            for h in range(HPG):
                pso = psop.tile([HD, BK], fp32, tag="o")
                pss = pssum.tile([P, BK], fp32, tag="sum")
                for i in range(ni):
                    r = i - 4 * j
                    t0 = P * max(r, 0)
                    pst = psmm.tile([P, BK], fp32, tag="mm")
                    nc.tensor.matmul(
                        pst[:, t0:],
                        k_sb[:, h, i * P:(i + 1) * P],
                        q_sb[:, h, t0:],
                        start=True, stop=True,
                    )
                    pt = ptp.tile([P, BK], bf, tag="pt")
                    nc.scalar.activation(pt[:, t0:], pst[:, t0:], Exp, scale=SCALE)
                    if r >= 0:
                        nc.vector.tensor_mul(
                            pt[:, t0:t0 + P], pt[:, t0:t0 + P], tri_sb[:]
                        )
                    nc.tensor.matmul(
                        pss[:, t0:], ones_sb[:], pt[:, t0:],
                        start=(i == 0), stop=(i == ni - 1),
                    )
                    nc.tensor.matmul(
                        pso[:, t0:], v_sb[:, i, h * HD:(h + 1) * HD], pt[:, t0:],
                        start=(i == 0), stop=(i == ni - 1),
                    )
                recip = workp.tile([P, BK], fp32, tag="recip")
                nc.vector.reciprocal(recip[:], pss[:])
                nc.vector.tensor_mul(ot_sb[:, h, :], pso[:], recip[:])

            # ================= Phase P(j): projection =================
            for tt in range(BK // P):
                for n in range(D // BK):
                    psy = psmm.tile([P, BK], fp32, tag="mm")
                    for h in range(HPG):
                        nc.tensor.matmul(
                            psy[:],
                            ot_sb[:, h, tt * P:(tt + 1) * P],
                            wp_sb[:, h, n * BK:(n + 1) * BK],
                            start=(h == 0), stop=(h == HPG - 1),
                        )
                    y = workp.tile([P, BK], fp32, tag="y")
                    nc.vector.tensor_copy(y[:], psy[:])
                    nc.sync.dma_start(
                        out_d[(j * 4 + tt) * P:(j * 4 + tt + 1) * P,
                              n * BK:(n + 1) * BK],
                        y[:],
                    )

    nc.compile()
    return nc


def _get_nc():
    if "nc" not in _NC_CACHE:
        _NC_CACHE["nc"] = _build_nc()
    return _NC_CACHE["nc"]


def _host_prep(x, Wqkv, Wproj, mask):
    """Build the 8 per-core input maps (host-side layout transforms)."""
    x = np.asarray(x, dtype=np.float32)
    Wqkv = np.asarray(Wqkv, dtype=np.float32)
    Wproj = np.asarray(Wproj, dtype=np.float32)
    mask = np.asarray(mask, dtype=np.float32)

    # RoPE tables (transposed layout [hd, T]); matches the standard
    # rotate-half RoPE with base 10000.
    inv_freq = 1.0 / (10000.0 ** (np.arange(0, HD, 2, dtype=np.float32) / HD))
    freqs = np.arange(T, dtype=np.float32)[:, None] * inv_freq[None, :]
    emb = np.concatenate([freqs, freqs], axis=-1)        # [T, 128]
    cosT = np.ascontiguousarray(np.cos(emb).T).astype(bf16)
    sinT = np.ascontiguousarray(np.sin(emb).T).astype(bf16)

    # rotate-half permutation (signs folded): out[d] = -in[d+64] (d<64),
    # +in[d-64] (d>=64); lhsT layout [K=i, M=d].
    rot = np.zeros((HD, HD), dtype=np.float32)
    for d in range(HD // 2):
        rot[d + HD // 2, d] = -1.0
    for d in range(HD // 2, HD):
        rot[d - HD // 2, d] = 1.0
    rot = rot.astype(bf16)

    # [s', t''] triangle for the diagonal 128x128 tile, from the real mask
    tri = (mask[:P, :P].T == 0.0).astype(bf16)
    ones = np.ones((P, P), dtype=bf16)

    in_maps = []
    for b in range(B):
        xT = np.ascontiguousarray(x[b].T).astype(bf16)
        for g in range(NG):
            heads = list(range(HPG * g, HPG * (g + 1)))
            wq = [Wqkv[:, h * HD:(h + 1) * HD] for h in heads]
            wk = [Wqkv[:, D + h * HD:D + (h + 1) * HD] for h in heads]
            wvl = [Wqkv[:, 2 * D + h * HD:2 * D + (h + 1) * HD] for h in heads]
            wqk = np.ascontiguousarray(
                np.concatenate(wq + wk, axis=1)).astype(bf16)
            wv = np.ascontiguousarray(np.concatenate(wvl, axis=1)).astype(bf16)
            wp = np.ascontiguousarray(
                Wproj[HPG * HD * g:HPG * HD * (g + 1), :]).astype(bf16)
            in_maps.append({
                "xt": xT, "wqk": wqk, "wv": wv, "wp": wp,
                "cos": cosT, "sin": sinT, "rot": rot, "tri": tri,
                "ones": ones,
            })
    return in_maps


def run(x, Wqkv, Wproj, bproj, mask, trace=False):
    """Run the SPMD kernel; returns (output, BassKernelResults)."""
    from concourse.bass_utils import run_bass_kernel_spmd

    nc = _get_nc()
    in_maps = _host_prep(x, Wqkv, Wproj, mask)
    res = run_bass_kernel_spmd(nc, in_maps, core_ids=list(range(B * NG)),
                               trace=trace)

    bproj = np.asarray(bproj, dtype=np.float32)
    out = np.zeros((B, T, D), dtype=np.float32)
    for b in range(B):
        acc = np.zeros((T, D), dtype=np.float32)
        for g in range(NG):
            acc += np.asarray(res.results[b * NG + g]["out"], dtype=np.float32)
        out[b] = acc + bproj[None, :]
    return out, res


def kernel(x, Wqkv, Wproj, bproj, mask):
    out, _ = run(x, Wqkv, Wproj, bproj, mask, trace=False)
    return out


# revision 5
# speedup vs baseline: 1.0484x; 1.0484x over previous
"""Trainium2 Bass kernel for nn_Attention_37641093382387.

Dense transformer attention block:
  qkv = x @ Wqkv; q,k + RoPE; causal softmax attention; out @ Wproj + bproj

Sharding: 8 cores = 2 batches x 4 head-groups (4 heads each).  Each core
computes its batch's partial output for its head group; host sums the 4
group partials per batch and adds the bias.

Per-core device pipeline (all matmuls bf16 -> f32 PSUM):
  - host passes x^T (pre-transposed, bf16) so no on-chip transposes needed
  - qT,kT computed in [hd, T] layout (lhsT=W block, rhs=xT);
    v in [T, hd] layout (lhsT=xT block, rhs=Wv)
  - RoPE rotate-half done via a +-1 permutation matmul on PE (DVE has no
    cross-partition path), then 3 DVE ops
  - attention computed transposed: ST[s,t] = kT_tile^T @ qT -> exp on ACT
    (scale folded into exp) -> PT bf16; row sums via all-ones matmul
    (replicated across partitions); OT accum = v_tile^T @ PT;
    normalization via DVE reciprocal+mul; causal handled by narrowing
    matmuls to the valid t-range + one 128x128 triangle mask multiply
  - proj: Y = OT^T blocks @ Wproj, f32 out
"""

import os
import sys

import numpy as np

for _p in ("/opt/trn_rl_repo",):
    if _p not in sys.path and os.path.isdir(_p):
        sys.path.insert(0, _p)

import ml_dtypes

bf16 = ml_dtypes.bfloat16

P = 128
T = 2048
D = 2048
HD = 128
NG = 4      # head groups
HPG = 4     # heads per group
B = 2
BK = 512    # t block
NB = T // BK          # 4 t-blocks
NKT = D // P          # 16 contraction chunks
NTT = T // P          # 16 t-tiles
SCALE = float(HD) ** -0.5

_NC_CACHE = {}


def _build_nc():
    import concourse.mybir as mybir
    from concourse import bacc
    from concourse.tile import TileContext

    fp32 = mybir.dt.float32
    bf = mybir.dt.bfloat16
    Exp = mybir.ActivationFunctionType.Exp

    nc = bacc.Bacc("TRN2", target_bir_lowering=False, debug=False,
                   num_devices=B * NG)

    xt_d = nc.declare_dram_parameter("xt", [D, T], bf, isOutput=False)
    wqk_d = nc.declare_dram_parameter("wqk", [D, 2 * HPG * HD], bf, isOutput=False)
    wv_d = nc.declare_dram_parameter("wv", [D, HPG * HD], bf, isOutput=False)
    wp_d = nc.declare_dram_parameter("wp", [HPG * HD, D], bf, isOutput=False)
    cos_d = nc.declare_dram_parameter("cos", [HD, T], bf, isOutput=False)
    sin_d = nc.declare_dram_parameter("sin", [HD, T], bf, isOutput=False)
    rot_d = nc.declare_dram_parameter("rot", [HD, HD], bf, isOutput=False)
    tri_d = nc.declare_dram_parameter("tri", [P, P], bf, isOutput=False)
    ones_d = nc.declare_dram_parameter("ones", [P, P], bf, isOutput=False)
    out_d = nc.declare_dram_parameter("out", [T, D], fp32, isOutput=True)

    xt_r = xt_d[:].rearrange("(kt p) t -> p kt t", p=P)

    with TileContext(nc) as tc, \
         tc.tile_pool(name="const", bufs=1) as constp, \
         tc.tile_pool(name="persist", bufs=1) as persistp, \
         tc.tile_pool(name="xt", bufs=2) as xtp, \
         tc.tile_pool(name="qblk", bufs=2) as qp, \
         tc.tile_pool(name="otblk", bufs=2) as otp, \
         tc.tile_pool(name="work", bufs=3) as workp, \
         tc.tile_pool(name="pt", bufs=4) as ptp, \
         tc.tile_pool(name="psmm", bufs=3, space="PSUM") as psmm, \
         tc.tile_pool(name="psrot", bufs=1, space="PSUM") as psrot, \
         tc.tile_pool(name="pssum", bufs=2, space="PSUM") as pssum, \
         tc.tile_pool(name="pso", bufs=2, space="PSUM") as psop:

        # ---- constants ----
        # Chunked loads so the first QKV matmuls only wait on small pieces,
        # and independent chunks spread across DMA queues.
        wqk_r = wqk_d[:].rearrange("(kt p) e -> p kt e", p=P)
        wv_r = wv_d[:].rearrange("(kt p) e -> p kt e", p=P)

        xt_sb0 = xtp.tile([P, NKT, BK], bf, tag="xt", name="xt_sb0")
        for c in range(4):
            nc.sync.dma_start(xt_sb0[:, 4 * c:4 * (c + 1), :],
                              xt_r[:, 4 * c:4 * (c + 1), 0:BK])
        wqk_sb = constp.tile([P, NKT, 2 * HPG * HD], bf)
        for kt in range(NKT):
            nc.sync.dma_start(wqk_sb[:, kt, :], wqk_r[:, kt, :])
        wv_sb = constp.tile([P, NKT, HPG * HD], bf)
        for c in range(4):
            nc.sync.dma_start(wv_sb[:, 4 * c:4 * (c + 1), :],
                              wv_r[:, 4 * c:4 * (c + 1), :])
        cos_sb = constp.tile([HD, T], bf)
        nc.sync.dma_start(cos_sb[:], cos_d[:])
        sin_sb = constp.tile([HD, T], bf)
        nc.sync.dma_start(sin_sb[:], sin_d[:])
        rot_sb = constp.tile([HD, HD], bf)
        nc.sync.dma_start(rot_sb[:], rot_d[:])
        tri_sb = constp.tile([P, P], bf)
        nc.sync.dma_start(tri_sb[:], tri_d[:])
        ones_sb = constp.tile([P, P], bf)
        nc.sync.dma_start(ones_sb[:], ones_d[:])
        wp_sb = constp.tile([P, HPG, D], bf)
        for c in range(4):
            nc.sync.dma_start(
                wp_sb[:, c, :],
                wp_d[:].rearrange("(ct p) f -> p ct f", p=P)[:, c, :])

        # ---- persistent tensors ----
        k_sb = persistp.tile([HD, HPG, T], bf)        # kT per head
        v_sb = persistp.tile([P, NTT, HPG * HD], bf)  # v  per t-tile

        for j in range(NB):
            tsl = slice(j * BK, (j + 1) * BK)

            # ================= Phase Q(j): qkT + RoPE, v =================
            if j == 0:
                xt_sb = xt_sb0
            else:
                xt_sb = xtp.tile([P, NKT, BK], bf, tag="xt",
                                 name=f"xt_sb{j}")
                for c in range(4):
                    nc.sync.dma_start(xt_sb[:, 4 * c:4 * (c + 1), :],
                                      xt_r[:, 4 * c:4 * (c + 1), tsl])

            q_sb = qp.tile([HD, HPG, BK], bf, tag="qblk")

            # rot matmul + RoPE for e-tile `e` are emitted after the QKV
            # matmul chain of e+1, so the PE never stalls on the ACT copy.
            def rope_tail(e, ps, raw):
                psr = psrot.tile([P, BK], fp32, tag="rot", name="psr")
                nc.tensor.matmul(psr[:], rot_sb[:], raw[:], start=True,
                                 stop=True)
                t1 = workp.tile([P, BK], fp32, tag="t1", name="t1")
                nc.vector.tensor_mul(t1[:], ps[:], cos_sb[:, tsl])
                t2 = workp.tile([P, BK], fp32, tag="t2", name="t2")
                nc.vector.tensor_mul(t2[:], psr[:], sin_sb[:, tsl])
                if e < HPG:
                    dst = q_sb[:, e, :]
                else:
                    dst = k_sb[:, e - HPG, tsl]
                nc.vector.tensor_add(dst, t1[:], t2[:])

            pending = None
            for e in range(2 * HPG):
                ps = psmm.tile([P, BK], fp32, tag="mm", name="ps_qk")
                for kt in range(NKT):
                    nc.tensor.matmul(
                        ps[:],
                        wqk_sb[:, kt, e * HD:(e + 1) * HD],
                        xt_sb[:, kt, :],
                        start=(kt == 0), stop=(kt == NKT - 1),
                    )
                raw = workp.tile([P, BK], bf, tag="raw", name="raw")
                nc.scalar.copy(raw[:], ps[:])
                if pending is not None:
                    rope_tail(*pending)
                pending = (e, ps, raw)
            rope_tail(*pending)

            for tt in range(BK // P):
                ps = psmm.tile([P, BK], fp32, tag="mm", name="ps_v")
                for kt in range(NKT):
                    nc.tensor.matmul(
                        ps[:],
                        xt_sb[:, kt, tt * P:(tt + 1) * P],
                        wv_sb[:, kt, :],
                        start=(kt == 0), stop=(kt == NKT - 1),
                    )
                nc.scalar.copy(v_sb[:, 4 * j + tt, :], ps[:])

            # ================= Phase A(j): attention =================
            ot_sb = otp.tile([HD, HPG, BK], bf, tag="otblk")
            ni = 4 * j + 4
            for h in range(HPG):
                pso = psop.tile([HD, BK], fp32, tag="o", name="pso")
                pss = pssum.tile([P, BK], fp32, tag="sum", name="pss")

                def st_stage(i):
                    r = i - 4 * j
                    t0 = P * max(r, 0)
                    pst = psmm.tile([P, BK], fp32, tag="mm", name="pst")
                    nc.tensor.matmul(
                        pst[:, t0:],
                        k_sb[:, h, i * P:(i + 1) * P],
                        q_sb[:, h, t0:],
                        start=True, stop=True,
                    )
                    pt = ptp.tile([P, BK], bf, tag="pt", name="pt")
                    nc.scalar.activation(pt[:, t0:], pst[:, t0:], Exp,
                                         scale=SCALE)
                    if r >= 0:
                        nc.gpsimd.tensor_mul(
                            pt[:, t0:t0 + P], pt[:, t0:t0 + P], tri_sb[:]
                        )
                    return (i, t0, pt)

                def pv_stage(i, t0, pt):
                    nc.tensor.matmul(
                        pss[:, t0:], ones_sb[:], pt[:, t0:],
                        start=(i == 0), stop=(i == ni - 1),
                    )
                    nc.tensor.matmul(
                        pso[:, t0:], v_sb[:, i, h * HD:(h + 1) * HD],
                        pt[:, t0:],
                        start=(i == 0), stop=(i == ni - 1),
                    )

                # software-pipeline: ST/exp runs 2 iterations ahead of the
                # sum/PV matmuls so the PE doesn't wait on ACT.
                from collections import deque
                fifo = deque()
                for i in range(ni):
                    fifo.append(st_stage(i))
                    if len(fifo) > 2:
                        pv_stage(*fifo.popleft())
                while fifo:
                    pv_stage(*fifo.popleft())

                recip = workp.tile([P, BK], fp32, tag="recip", name="recip")
                nc.vector.reciprocal(recip[:], pss[:])
                nc.vector.tensor_mul(ot_sb[:, h, :], pso[:], recip[:])

            # ================= Phase P(j): projection =================
            for tt in range(BK // P):
                for n in range(D // BK):
                    psy = psmm.tile([P, BK], fp32, tag="mm", name="psy")
                    for h in range(HPG):
                        nc.tensor.matmul(
                            psy[:],
                            ot_sb[:, h, tt * P:(tt + 1) * P],
                            wp_sb[:, h, n * BK:(n + 1) * BK],
                            start=(h == 0), stop=(h == HPG - 1),
                        )
                    y = workp.tile([P, BK], fp32, tag="y", name="y")
                    nc.scalar.copy(y[:], psy[:])
                    nc.sync.dma_start(
                        out_d[(j * 4 + tt) * P:(j * 4 + tt + 1) * P,
                              n * BK:(n + 1) * BK],
                        y[:],
                    )

    nc.compile()
    return nc


def _get_nc():
    if "nc" not in _NC_CACHE:
        _NC_CACHE["nc"] = _build_nc()
    return _NC_CACHE["nc"]


def _host_prep(x, Wqkv, Wproj, mask):
    """Build the 8 per-core input maps (host-side layout transforms)."""
    x = np.asarray(x, dtype=np.float32)
    Wqkv = np.asarray(Wqkv, dtype=np.float32)
    Wproj = np.asarray(Wproj, dtype=np.float32)
    mask = np.asarray(mask, dtype=np.float32)

    # RoPE tables (transposed layout [hd, T]); matches the standard
    # rotate-half RoPE with base 10000.
    inv_freq = 1.0 / (10000.0 ** (np.arange(0, HD, 2, dtype=np.float32) / HD))
    freqs = np.arange(T, dtype=np.float32)[:, None] * inv_freq[None, :]
    emb = np.concatenate([freqs, freqs], axis=-1)        # [T, 128]
    cosT = np.ascontiguousarray(np.cos(emb).T).astype(bf16)
    sinT = np.ascontiguousarray(np.sin(emb).T).astype(bf16)

    # rotate-half permutation (signs folded): out[d] = -in[d+64] (d<64),
    # +in[d-64] (d>=64); lhsT layout [K=i, M=d].
    rot = np.zeros((HD, HD), dtype=np.float32)
    for d in range(HD // 2):
        rot[d + HD // 2, d] = -1.0
    for d in range(HD // 2, HD):
        rot[d - HD // 2, d] = 1.0
    rot = rot.astype(bf16)

    # [s', t''] triangle for the diagonal 128x128 tile, from the real mask
    tri = (mask[:P, :P].T == 0.0).astype(bf16)
    ones = np.ones((P, P), dtype=bf16)

    in_maps = []
    for b in range(B):
        xT = np.ascontiguousarray(x[b].T).astype(bf16)
        for g in range(NG):
            heads = list(range(HPG * g, HPG * (g + 1)))
            wq = [Wqkv[:, h * HD:(h + 1) * HD] for h in heads]
            wk = [Wqkv[:, D + h * HD:D + (h + 1) * HD] for h in heads]
            wvl = [Wqkv[:, 2 * D + h * HD:2 * D + (h + 1) * HD] for h in heads]
            wqk = np.ascontiguousarray(
                np.concatenate(wq + wk, axis=1)).astype(bf16)
            wv = np.ascontiguousarray(np.concatenate(wvl, axis=1)).astype(bf16)
            wp = np.ascontiguousarray(
                Wproj[HPG * HD * g:HPG * HD * (g + 1), :]).astype(bf16)
            in_maps.append({
                "xt": xT, "wqk": wqk, "wv": wv, "wp": wp,
                "cos": cosT, "sin": sinT, "rot": rot, "tri": tri,
                "ones": ones,
            })
    return in_maps


def run(x, Wqkv, Wproj, bproj, mask, trace=False):
    """Run the SPMD kernel; returns (output, BassKernelResults)."""
    from concourse.bass_utils import run_bass_kernel_spmd

    nc = _get_nc()
    in_maps = _host_prep(x, Wqkv, Wproj, mask)
    res = run_bass_kernel_spmd(nc, in_maps, core_ids=list(range(B * NG)),
                               trace=trace)

    bproj = np.asarray(bproj, dtype=np.float32)
    out = np.zeros((B, T, D), dtype=np.float32)
    for b in range(B):
        acc = np.zeros((T, D), dtype=np.float32)
        for g in range(NG):
            acc += np.asarray(res.results[b * NG + g]["out"], dtype=np.float32)
        out[b] = acc + bproj[None, :]
    return out, res


def kernel(x, Wqkv, Wproj, bproj, mask):
    out, _ = run(x, Wqkv, Wproj, bproj, mask, trace=False)
    return out
